# revision 3
# baseline (speedup 1.0000x reference)
"""Bass/Trainium2 kernel for nn_LongformerAttention_44315472560501.

The reference op reduces to:
  output            = hidden_states                  # [B,S,H] identity copy
  attention_weights = zeros([B,S,S], f32)

Pure memory problem. Sharding (per the hint): data-parallel over batch (B=2)
and sequence-sharded over S in 4 chunks -> 8 cores, each handling a
[1024, 1024] copy slice and a [1024, 4096] zeros slice.

Per-core device program:
  - DRAM->DRAM DMA copy of the hidden_states shard (no SBUF round trip)
  - memset an SBUF tile to 0 once, then DMA-broadcast it over the
    attention-weights shard
"""

import numpy as np

import concourse.bass as bass
import concourse.mybir as mybir
import concourse.tile as tile
from concourse.bass_utils import run_bass_kernel_spmd

B, S, H = 2, 4096, 1024
N_CORES = 8
SEQ_SHARDS = N_CORES // B  # 4 sequence chunks per batch element
SS = S // SEQ_SHARDS       # 1024 sequence rows per core

FP32 = mybir.dt.float32


def _build_program(nz_per_ring: int = 2) -> bass.Bass:
    """Raw-Bass program (no TileContext): per ring (sync=SP and scalar=ACT
    HWDGE), one 2 MiB DRAM->DRAM copy DMA plus nz_per_ring broadcast
    zero-fill DMAs reading a memset SBUF tile through a stride-0 free dim.
    One shared semaphore for all DMA completions keeps the tail to a single
    wait per engine (TileContext's drain aggregates every sem lane and
    exceeds the HW sync-wait limit)."""
    nc = bass.Bass()
    x = nc.dram_tensor("x", [SS, H], FP32, kind="ExternalInput")
    y = nc.dram_tensor("y", [SS, H], FP32, kind="ExternalOutput")
    w = nc.dram_tensor("w", [SS, S], FP32, kind="ExternalOutput")

    half = SS // 2
    nz = 2 * nz_per_ring
    blk = SS // nz
    total_dmas = 2 + nz

    with (
        nc.sbuf_tensor("zt", [128, S], FP32) as zt,
        nc.Block(no_gpsimd_drain=True) as block,
        nc.semaphore("zsem") as zsem,
        nc.semaphore("dsem") as dsem,
    ):

        @block.vector
        def _(vector):
            vector.memset(zt[:], 0.0).then_inc(zsem, 1)

        def ring(eng, ring_idx):
            eng.dma_start(
                out=y[ring_idx * half : (ring_idx + 1) * half, :],
                in_=x[ring_idx * half : (ring_idx + 1) * half, :],
            ).then_inc(dsem, 16)
            eng.wait_ge(zsem, 1)
            for i in range(ring_idx * nz_per_ring, (ring_idx + 1) * nz_per_ring):
                w3 = w[i * blk : (i + 1) * blk, :].rearrange(
                    "(n p) s -> p n s", p=128
                )
                eng.dma_start(
                    out=w3,
                    in_=zt[:].unsqueeze(1).broadcast_to([128, blk // 128, S]),
                ).then_inc(dsem, 16)
            eng.wait_ge(dsem, 16 * total_dmas)

        @block.sync
        def _(sync):
            ring(sync, 0)

        @block.scalar
        def _(scalar):
            ring(scalar, 1)

    return nc


def _run(hidden_states: np.ndarray, **run_kwargs):
    hs = np.ascontiguousarray(np.asarray(hidden_states, dtype=np.float32))
    assert hs.shape == (B, S, H), hs.shape

    nc = _build_program()
    in_maps = []
    for c in range(N_CORES):
        b, q = divmod(c, SEQ_SHARDS)
        in_maps.append({"x": hs[b, q * SS : (q + 1) * SS, :]})

    res = run_bass_kernel_spmd(nc, in_maps, list(range(N_CORES)), **run_kwargs)

    out = np.empty((B, S, H), np.float32)
    attn = np.empty((B, S, S), np.float32)
    for c in range(N_CORES):
        b, q = divmod(c, SEQ_SHARDS)
        out[b, q * SS : (q + 1) * SS, :] = res.results[c]["y"]
        attn[b, q * SS : (q + 1) * SS, :] = res.results[c]["w"]
    return (out, attn), res


def kernel(hidden_states, global_attention_mask=None):
    outputs, _ = _run(hidden_states)
    return outputs
